# revision 5
# baseline (speedup 1.0000x reference)
"""Trainium2 Bass kernel for the BLIPv2 agent-attention block.

Full (unsharded) inputs in, full outputs out. Data-parallel over the batch
axis across 8 NeuronCores; parameters replicated.

Per-core program (64 batch items, 65 tokens, dim 768):
  Phase A: LN1 -> qkv (f32r matmuls) -> per-item agent attention (bf16
           matmuls) + depthwise 3x3 conv (DVE) -> proj -> xa = x + att
           (bounced to DRAM).
  Phase B: LN2 -> fc1 (f32r) -> exact gelu (ACT) -> fc2 (bf16) -> out =
           xa + mlp.
Layout: token-major for LayerNorm / residuals, feature-major for matmuls,
PE transposes to convert.
"""

import sys
from contextlib import ExitStack

import numpy as np

sys.path.insert(0, "/opt/trn_rl_repo")

DIM = 768
KC = 6            # DIM / 128
NTOK = 65
AG = 49
MLP = 3072
MC = 24           # MLP / 128
NCORES = 8
NITEMS = 64       # batch items per core
NB = 16           # items per sub-batch
NSB = NITEMS // NB
TSB = NB * NTOK   # tokens per sub-batch (1040)
TOKS = NITEMS * NTOK  # tokens per core (4160)
EPS = 1e-5
SCALE4 = (DIM ** -0.5) / 4.0  # folds the 2x2-avg-pool 1/4 into the softmax scale

_PROG = None


def _ntiles(total, step):
    out = []
    o = 0
    while o < total:
        out.append((o, min(step, total - o)))
        o += step
    return out


def _build_program():
    import concourse.bass as bass
    import concourse.mybir as mybir
    from concourse import bacc
    from concourse.tile import TileContext
    from concourse.masks import make_identity

    f32 = mybir.dt.float32
    f32r = mybir.dt.float32r
    bf16 = mybir.dt.bfloat16
    AF = mybir.ActivationFunctionType
    OP = mybir.AluOpType
    AX = mybir.AxisListType

    nc = bacc.Bacc("TRN2", target_bir_lowering=False, debug=False,
                   num_devices=NCORES)

    x_d = nc.dram_tensor("x", [TOKS, DIM], f32, kind="ExternalInput")
    wqkv_d = nc.dram_tensor("wqkvT", [DIM, 3 * DIM], f32, kind="ExternalInput")
    wproj_d = nc.dram_tensor("wprojT", [DIM, DIM], f32, kind="ExternalInput")
    fc1_d = nc.dram_tensor("fc1T", [DIM, MLP], f32, kind="ExternalInput")
    fc2_d = nc.dram_tensor("fc2T", [MLP, DIM], f32, kind="ExternalInput")
    ln1g_d = nc.dram_tensor("ln1g", [128, KC], f32, kind="ExternalInput")
    ln1b_d = nc.dram_tensor("ln1b", [128, KC], f32, kind="ExternalInput")
    ln2g_d = nc.dram_tensor("ln2g", [128, KC], f32, kind="ExternalInput")
    ln2b_d = nc.dram_tensor("ln2b", [128, KC], f32, kind="ExternalInput")
    bproj_d = nc.dram_tensor("bproj", [128, KC], f32, kind="ExternalInput")
    fc1b_d = nc.dram_tensor("fc1b", [128, MC], f32, kind="ExternalInput")
    fc2b_d = nc.dram_tensor("fc2b", [128, KC], f32, kind="ExternalInput")
    dwcw_d = nc.dram_tensor("dwcw", [128, KC * 9], f32, kind="ExternalInput")
    dwcb_d = nc.dram_tensor("dwcb", [128, KC], f32, kind="ExternalInput")
    pbc_d = nc.dram_tensor("pbc", [AG, NTOK], f32, kind="ExternalInput")
    abc_d = nc.dram_tensor("abc", [NTOK, AG], f32, kind="ExternalInput")

    xa_d = nc.dram_tensor("xa", [TOKS, DIM], f32)
    xout_d = nc.dram_tensor("xout", [TOKS, DIM], f32, kind="ExternalOutput")
    rep_d = nc.dram_tensor("rep", [NITEMS, AG, NTOK], f32, kind="ExternalOutput")

    xap = x_d.ap()
    xaap = xa_d.ap()
    xoap = xout_d.ap()
    repap = rep_d.ap()

    with TileContext(nc) as tc, ExitStack() as _stk:
        cns = _stk.enter_context(tc.tile_pool(name="consts", bufs=1))
        identf = cns.tile([128, 128], f32)
        make_identity(nc, identf[:, :])
        identr = cns.tile([128, 128], f32r)
        nc.vector.tensor_copy(identr[:, :], identf[:, :])
        identb = cns.tile([128, 128], bf16)
        make_identity(nc, identb[:, :])
        eps_t = cns.tile([128, 1], f32)
        nc.vector.memset(eps_t[:, :], EPS)

        def _load_const(dram, shape):
            t = cns.tile(shape, f32, name=dram.name + "_t")
            nc.sync.dma_start(out=t[...], in_=dram.ap())
            return t

        ln1g_t = _load_const(ln1g_d, [128, KC])
        ln1b_t = _load_const(ln1b_d, [128, KC])
        ln2g_t = _load_const(ln2g_d, [128, KC])
        ln2b_t = _load_const(ln2b_d, [128, KC])
        bproj_t = _load_const(bproj_d, [128, KC])
        fc1b_t = _load_const(fc1b_d, [128, MC])
        fc2b_t = _load_const(fc2b_d, [128, KC])
        dwcw_t = _load_const(dwcw_d, [128, KC * 9])
        dwcb_t = _load_const(dwcb_d, [128, KC])
        pbc_t = _load_const(pbc_d, [AG, NTOK])
        abc_t = _load_const(abc_d, [NTOK, AG])

        # ---------------- Phase A ----------------
        pwa = tc.tile_pool(name="pwa", bufs=1)
        pbig = tc.tile_pool(name="pbig", bufs=1)
        pwk = tc.tile_pool(name="pwk", bufs=1)
        pps = tc.tile_pool(name="pps", bufs=1, space="PSUM")
        with pwa as wA, pbig as big, pwk as wk, pps as ps:
            # load weights, converting f32 -> f32r on DVE via a staging tile
            wqkv_t = wA.tile([128, KC, 3 * DIM], f32r)
            wproj_t = wA.tile([128, KC, DIM], f32r)
            wq_r = wqkv_d.ap().rearrange("(k p) m -> k p m", p=128)
            wp_r = wproj_d.ap().rearrange("(k p) m -> k p m", p=128)
            for k in range(KC):
                for pc in range(3):
                    st = wk.tile([128, DIM], f32, tag="stage", bufs=2)
                    nc.sync.dma_start(out=st[:, :],
                                      in_=wq_r[k][:, pc * DIM:(pc + 1) * DIM])
                    nc.vector.tensor_copy(
                        wqkv_t[:, k, pc * DIM:(pc + 1) * DIM], st[:, :])
                st2 = wk.tile([128, DIM], f32, tag="stage", bufs=2)
                nc.sync.dma_start(out=st2[:, :], in_=wp_r[k])
                nc.vector.tensor_copy(wproj_t[:, k, :], st2[:, :])

            for sb in range(NSB):
                t0sb = sb * TSB
                # -- A-i: LN1 + transpose to feature-major (f32r) --
                xhf = big.tile([128, KC, TSB], f32r, tag="xy", name="xhf")
                for (tt0, tp) in _ntiles(TSB, 128):
                    x_tm = wk.tile([128, DIM], f32, tag="x_tm", bufs=2)
                    nc.sync.dma_start(out=x_tm[:tp, :],
                                      in_=xap[t0sb + tt0: t0sb + tt0 + tp, :])
                    stats = wk.tile([128, 3, 6], f32, tag="stats", bufs=2)
                    xg = x_tm[:tp, :].rearrange("p (g d) -> p g d", g=3)
                    for g in range(3):
                        nc.vector.bn_stats(out=stats[:tp, g, :], in_=xg[:, g, :])
                    mv = wk.tile([128, 2], f32, tag="mv", bufs=2)
                    nc.vector.bn_aggr(out=mv[:tp, :], in_=stats[:tp, :, :])
                    rstd = wk.tile([128, 1], f32, tag="rstd", bufs=2)
                    nc.scalar.activation(rstd[:tp, :], mv[:tp, 1:2], AF.Sqrt,
                                         bias=eps_t[:tp, :])
                    nc.vector.reciprocal(rstd[:tp, :], rstd[:tp, :])
                    xh_tm = wk.tile([128, DIM], f32r, tag="xh_tm", bufs=2)
                    nc.vector.tensor_scalar(out=xh_tm[:tp, :], in0=x_tm[:tp, :],
                                            scalar1=mv[:tp, 0:1],
                                            scalar2=rstd[:tp, :],
                                            op0=OP.subtract, op1=OP.mult)
                    for k in range(KC):
                        tps = ps.tile([128, 128], f32r, tag="tp", bufs=2,
                                      name="tps")
                        nc.tensor.transpose(tps[:, :tp],
                                            xh_tm[:tp, k * 128:(k + 1) * 128],
                                            identr[:tp, :tp])
                        nc.vector.tensor_scalar(
                            out=xhf[:, k, tt0:tt0 + tp], in0=tps[:, :tp],
                            scalar1=ln1g_t[:, k:k + 1],
                            scalar2=ln1b_t[:, k:k + 1],
                            op0=OP.mult, op1=OP.add)

                # -- A-ii: qkv matmuls (f32r) -> q/k/v bf16 --
                qf = big.tile([128, KC, TSB], bf16, tag="qf", name="qf")
                kf = big.tile([128, KC, TSB], bf16, tag="kf", name="kf")
                vf = big.tile([128, KC, TSB], bf16, tag="vf", name="vf")
                for (n0, nn) in _ntiles(TSB, 512):
                    for m in range(3 * KC):
                        mm = ps.tile([128, 512], f32, tag="mm", bufs=2,
                                     name="mmq")
                        for k in range(KC):
                            nc.tensor.matmul(
                                mm[:, :nn],
                                wqkv_t[:, k, m * 128:(m + 1) * 128],
                                xhf[:, k, n0:n0 + nn],
                                start=(k == 0), stop=(k == KC - 1))
                        dst = (qf, kf, vf)[m // KC]
                        nc.any.tensor_copy(dst[:, m % KC, n0:n0 + nn],
                                           mm[:, :nn])

                # -- A-iii: depthwise 3x3 conv (batched) --
                dw = big.tile([128, KC, NB, NTOK], bf16, tag="dwproj",
                              name="dw")
                nc.vector.memset(dw[...], 0.0)
                for k in range(KC):
                    vsp = vf[:, k, :].rearrange("p (i n) -> p i n", i=NB)
                    vgrid = vsp[:, :, 1:NTOK].rearrange(
                        "p i (y x) -> p i y x", y=8)
                    dgrid = dw[:, k, :, 1:NTOK].rearrange(
                        "p i (y x) -> p i y x", y=8)
                    for dy in (-1, 0, 1):
                        for dx in (-1, 0, 1):
                            ny, nx = 8 - abs(dy), 8 - abs(dx)
                            oy, ox = max(0, -dy), max(0, -dx)
                            iy, ix = max(0, dy), max(0, dx)
                            tap = (dy + 1) * 3 + (dx + 1)
                            tmp = wk.tile([128, NB, 8, 8], bf16, tag="dwtmp",
                                          bufs=2)
                            nc.vector.tensor_scalar(
                                out=tmp[:, :, :ny, :nx],
                                in0=vgrid[:, :, iy:iy + ny, ix:ix + nx],
                                scalar1=dwcw_t[:, k * 9 + tap:k * 9 + tap + 1],
                                scalar2=None, op0=OP.mult)
                            nc.vector.tensor_add(
                                out=dgrid[:, :, oy:oy + ny, ox:ox + nx],
                                in0=dgrid[:, :, oy:oy + ny, ox:ox + nx],
                                in1=tmp[:, :, :ny, :nx])
                    nc.vector.tensor_scalar(
                        out=dw[:, k, :, 1:NTOK], in0=dw[:, k, :, 1:NTOK],
                        scalar1=dwcb_t[:, k:k + 1], scalar2=None, op0=OP.add)

                # -- A-iv: per-item agent attention -> y (f32r, reuses xhf) --
                yf = big.tile([128, KC, TSB], f32r, tag="xy", name="yf")
                for i in range(NB):
                    ci = i * NTOK
                    gi = sb * NB + i
                    qs = qf[:, :, ci:ci + NTOK]
                    kss = kf[:, :, ci:ci + NTOK]

                    # 2x2 mean pool of q spatial grid -> 4*agent (bf16)
                    qgrid = qf[:, :, ci + 1:ci + NTOK].rearrange(
                        "p k (y x) -> p k y x", y=8)
                    t1 = wk.tile([128, KC, 7, 8], bf16, tag="t1", bufs=2)
                    nc.vector.tensor_add(out=t1[...],
                                         in0=qgrid[:, :, 0:7, :],
                                         in1=qgrid[:, :, 1:8, :])
                    agent = wk.tile([128, KC, AG], bf16, tag="agent", bufs=2)
                    ag4 = agent[:, :, :].rearrange("p k (y x) -> p k y x", y=7)
                    nc.vector.tensor_add(out=ag4[...],
                                         in0=t1[:, :, :, 0:7],
                                         in1=t1[:, :, :, 1:8])

                    # s1 = 4agent @ k^T   [49, 65]
                    s1p = ps.tile([AG, NTOK], f32, tag="at", bufs=4,
                                  name="s1p")
                    for k in range(KC):
                        nc.tensor.matmul(s1p[:, :], agent[:, k, :],
                                         kss[:, k, :],
                                         start=(k == 0), stop=(k == KC - 1))
                    s1s = wk.tile([AG, NTOK], f32, tag="s1s", bufs=2)
                    nc.vector.tensor_add(out=s1s[:, :], in0=s1p[:, :],
                                         in1=pbc_t[:, :])
                    nm1 = wk.tile([AG, 1], f32, tag="nm1", bufs=2)
                    nc.vector.tensor_reduce(nm1[:, :], s1s[:, :], axis=AX.X,
                                            op=OP.max, negate=True)
                    nm1c = wk.tile([AG, 1], f32, tag="nm1c", bufs=2)
                    nc.vector.tensor_scalar(out=nm1c[:, :], in0=nm1[:, :],
                                            scalar1=SCALE4, scalar2=None,
                                            op0=OP.mult)
                    a1 = wk.tile([AG, NTOK + 1], f32, tag="a1", bufs=2)
                    nc.scalar.activation(a1[:, :NTOK], s1s[:, :], AF.Exp,
                                         bias=nm1c[:, :], scale=SCALE4,
                                         accum_out=a1[:, NTOK:NTOK + 1])
                    rr1 = wk.tile([AG, 1], f32, tag="rr1", bufs=2)
                    nc.vector.reciprocal(rr1[:, :], a1[:, NTOK:NTOK + 1])
                    nc.vector.tensor_scalar(out=a1[:, :NTOK], in0=a1[:, :NTOK],
                                            scalar1=rr1[:, :], scalar2=None,
                                            op0=OP.mult)
                    nc.sync.dma_start(out=repap[gi], in_=a1[:, :NTOK])

                    a1tp = ps.tile([NTOK, AG], f32, tag="at", bufs=4,
                                   name="a1tp")
                    nc.tensor.transpose(a1tp[:, :], a1[:, :NTOK],
                                        identf[:AG, :AG])
                    a1t = wk.tile([NTOK, AG], bf16, tag="a1t", bufs=2)
                    nc.any.tensor_copy(a1t[:, :], a1tp[:, :])

                    # s2 = q @ agent^T  [65, 49]
                    s2p = ps.tile([NTOK, AG], f32, tag="at", bufs=4,
                                  name="s2p")
                    for k in range(KC):
                        nc.tensor.matmul(s2p[:, :], qs[:, k, :],
                                         agent[:, k, :],
                                         start=(k == 0), stop=(k == KC - 1))
                    s2s = wk.tile([NTOK, AG], f32, tag="s2s", bufs=2)
                    nc.vector.tensor_add(out=s2s[:, :], in0=s2p[:, :],
                                         in1=abc_t[:, :])
                    nm2 = wk.tile([NTOK, 1], f32, tag="nm2", bufs=2)
                    nc.vector.tensor_reduce(nm2[:, :], s2s[:, :], axis=AX.X,
                                            op=OP.max, negate=True)
                    nm2c = wk.tile([NTOK, 1], f32, tag="nm2c", bufs=2)
                    nc.vector.tensor_scalar(out=nm2c[:, :], in0=nm2[:, :],
                                            scalar1=SCALE4, scalar2=None,
                                            op0=OP.mult)
                    qa = wk.tile([NTOK, AG + 1], f32, tag="qa", bufs=2)
                    nc.scalar.activation(qa[:, :AG], s2s[:, :], AF.Exp,
                                         bias=nm2c[:, :], scale=SCALE4,
                                         accum_out=qa[:, AG:AG + 1])
                    rr2 = wk.tile([NTOK, 1], f32, tag="rr2", bufs=2)
                    nc.vector.reciprocal(rr2[:, :], qa[:, AG:AG + 1])
                    nc.vector.tensor_scalar(out=qa[:, :AG], in0=qa[:, :AG],
                                            scalar1=rr2[:, :], scalar2=None,
                                            op0=OP.mult)
                    qatp = ps.tile([AG, NTOK], f32, tag="at", bufs=4,
                                   name="qatp")
                    nc.tensor.transpose(qatp[:, :], qa[:, :AG],
                                        identf[:NTOK, :NTOK])
                    qat = wk.tile([AG, NTOK], bf16, tag="qat", bufs=2)
                    nc.any.tensor_copy(qat[:, :], qatp[:, :])

                    # v_tm: per-item transpose of v to token-major (bf16)
                    vtm = wk.tile([128, DIM], bf16, tag="vtm", bufs=2)
                    for k in range(KC):
                        vtp = ps.tile([NTOK, 128], bf16, tag="at", bufs=4,
                                      name="vtp")
                        nc.tensor.transpose(vtp[:, :], vf[:, k, ci:ci + NTOK],
                                            identb[:, :])
                        nc.any.tensor_copy(
                            vtm[:NTOK, k * 128:(k + 1) * 128], vtp[:, :])

                    # agent_v = A1 @ v  [49, 768]
                    av = wk.tile([AG, DIM], bf16, tag="av", bufs=2)
                    avp1 = ps.tile([AG, 512], f32, tag="at", bufs=4,
                                   name="avp1")
                    nc.tensor.matmul(avp1[:, :], a1t[:, :], vtm[:NTOK, 0:512],
                                     start=True, stop=True)
                    nc.any.tensor_copy(av[:, 0:512], avp1[:, :])
                    avp2 = ps.tile([AG, 256], f32, tag="at", bufs=4,
                                   name="avp2")
                    nc.tensor.matmul(avp2[:, :], a1t[:, :], vtm[:NTOK, 512:768],
                                     start=True, stop=True)
                    nc.any.tensor_copy(av[:, 512:768], avp2[:, :])

                    # out^T = agent_v^T @ q_attn^T  [768, 65] + dw -> y
                    op_ = ps.tile([128, KC, NTOK], f32, tag="at", bufs=4,
                                  name="op_")
                    for k in range(KC):
                        nc.tensor.matmul(op_[:, k, :],
                                         av[:, k * 128:(k + 1) * 128],
                                         qat[:, :], start=True, stop=True)
                    nc.vector.tensor_add(out=yf[:, :, ci:ci + NTOK],
                                         in0=op_[:, :, :],
                                         in1=dw[:, :, i, :])

                # -- A-v: proj (f32r) + bias -> proj_fm (bf16) --
                pjf = big.tile([128, KC, TSB], bf16, tag="dwproj", name="pjf")
                for (n0, nn) in _ntiles(TSB, 512):
                    for m in range(KC):
                        mm = ps.tile([128, 512], f32, tag="mm", bufs=2,
                                     name="mmp")
                        for k in range(KC):
                            nc.tensor.matmul(
                                mm[:, :nn],
                                wproj_t[:, k, m * 128:(m + 1) * 128],
                                yf[:, k, n0:n0 + nn],
                                start=(k == 0), stop=(k == KC - 1))
                        nc.vector.tensor_scalar(
                            out=pjf[:, m, n0:n0 + nn], in0=mm[:, :nn],
                            scalar1=bproj_t[:, m:m + 1], scalar2=None,
                            op0=OP.add)

                # -- A-vi: xa = x + att (token-major), bounce to DRAM --
                for (tt0, tp) in _ntiles(TSB, 128):
                    x2_tm = wk.tile([128, DIM], f32, tag="x2_tm", bufs=2)
                    nc.sync.dma_start(out=x2_tm[:tp, :],
                                      in_=xap[t0sb + tt0: t0sb + tt0 + tp, :])
                    xa_tm = wk.tile([128, DIM], f32, tag="xa_tm", bufs=2)
                    for k in range(KC):
                        tb = ps.tile([128, 128], bf16, tag="tp", bufs=2,
                                     name="tb")
                        nc.tensor.transpose(tb[:tp, :],
                                            pjf[:, k, tt0:tt0 + tp],
                                            identb[:, :])
                        nc.vector.tensor_add(
                            out=xa_tm[:tp, k * 128:(k + 1) * 128],
                            in0=tb[:tp, :],
                            in1=x2_tm[:tp, k * 128:(k + 1) * 128])
                    nc.sync.dma_start(
                        out=xaap[t0sb + tt0: t0sb + tt0 + tp, :],
                        in_=xa_tm[:tp, :])

        # ---------------- Phase B ----------------
        pwb = tc.tile_pool(name="pwb", bufs=1)
        pbig2 = tc.tile_pool(name="pbig2", bufs=1)
        pwk2 = tc.tile_pool(name="pwk2", bufs=1)
        pps2 = tc.tile_pool(name="pps2", bufs=1, space="PSUM")
        with pwb as wB, pbig2 as big2, pwk2 as wk2, pps2 as ps2:
            fc1_t = wB.tile([128, KC, MLP], f32r)
            fc2_t = wB.tile([128, MC, DIM], bf16)
            f1_r = fc1_d.ap().rearrange("(k p) m -> k p m", p=128)
            f2_r = fc2_d.ap().rearrange("(k p) m -> k p m", p=128)
            for k in range(KC):
                for pc in range(4):
                    st = wk2.tile([128, DIM], f32, tag="stage", bufs=2)
                    nc.sync.dma_start(out=st[:, :],
                                      in_=f1_r[k][:, pc * DIM:(pc + 1) * DIM])
                    nc.vector.tensor_copy(
                        fc1_t[:, k, pc * DIM:(pc + 1) * DIM], st[:, :])
            for k in range(MC):
                st2 = wk2.tile([128, DIM], f32, tag="stage", bufs=2)
                nc.sync.dma_start(out=st2[:, :], in_=f2_r[k])
                nc.vector.tensor_copy(fc2_t[:, k, :], st2[:, :])

            for (g0, gn) in _ntiles(TOKS, 512):
                xhf2 = big2.tile([128, KC, 512], f32r, tag="xhf2",
                                 name="xhf2")
                for (tt0, tp) in _ntiles(gn, 128):
                    xa_tm = wk2.tile([128, DIM], f32, tag="xa_tm", bufs=3)
                    nc.sync.dma_start(out=xa_tm[:tp, :],
                                      in_=xaap[g0 + tt0: g0 + tt0 + tp, :])
                    stats = wk2.tile([128, 3, 6], f32, tag="stats", bufs=2)
                    xg = xa_tm[:tp, :].rearrange("p (g d) -> p g d", g=3)
                    for g in range(3):
                        nc.vector.bn_stats(out=stats[:tp, g, :],
                                           in_=xg[:, g, :])
                    mv = wk2.tile([128, 2], f32, tag="mv", bufs=2)
                    nc.vector.bn_aggr(out=mv[:tp, :], in_=stats[:tp, :, :])
                    rstd = wk2.tile([128, 1], f32, tag="rstd", bufs=2)
                    nc.scalar.activation(rstd[:tp, :], mv[:tp, 1:2], AF.Sqrt,
                                         bias=eps_t[:tp, :])
                    nc.vector.reciprocal(rstd[:tp, :], rstd[:tp, :])
                    xh_tm = wk2.tile([128, DIM], f32r, tag="xh_tm", bufs=2)
                    nc.vector.tensor_scalar(out=xh_tm[:tp, :],
                                            in0=xa_tm[:tp, :],
                                            scalar1=mv[:tp, 0:1],
                                            scalar2=rstd[:tp, :],
                                            op0=OP.subtract, op1=OP.mult)
                    for k in range(KC):
                        tps = ps2.tile([128, 128], f32r, tag="tp", bufs=2,
                                       name="tps2")
                        nc.tensor.transpose(tps[:, :tp],
                                            xh_tm[:tp, k * 128:(k + 1) * 128],
                                            identr[:tp, :tp])
                        nc.vector.tensor_scalar(
                            out=xhf2[:, k, tt0:tt0 + tp], in0=tps[:, :tp],
                            scalar1=ln2g_t[:, k:k + 1],
                            scalar2=ln2b_t[:, k:k + 1],
                            op0=OP.mult, op1=OP.add)

                hts = []
                for m in range(MC):
                    mm = ps2.tile([128, 512], f32, tag="mm", bufs=2,
                                  name="mmf1")
                    for k in range(KC):
                        nc.tensor.matmul(mm[:, :gn],
                                         fc1_t[:, k, m * 128:(m + 1) * 128],
                                         xhf2[:, k, :gn],
                                         start=(k == 0), stop=(k == KC - 1))
                    ht = big2.tile([128, 512], bf16, tag="h", bufs=26,
                                   name=f"h{m}")
                    nc.scalar.activation(ht[:, :gn], mm[:, :gn], AF.Gelu,
                                         bias=fc1b_t[:, m:m + 1])
                    hts.append(ht)

                mlpf = big2.tile([128, KC, 512], bf16, tag="mlpf",
                                 name="mlpf")
                for m in range(KC):
                    mm2 = ps2.tile([128, 512], f32, tag="mm2", bufs=2,
                                   name="mmf2")
                    for k in range(MC):
                        nc.tensor.matmul(mm2[:, :gn],
                                         fc2_t[:, k, m * 128:(m + 1) * 128],
                                         hts[k][:, :gn],
                                         start=(k == 0), stop=(k == MC - 1))
                    nc.vector.tensor_scalar(
                        out=mlpf[:, m, :gn], in0=mm2[:, :gn],
                        scalar1=fc2b_t[:, m:m + 1], scalar2=None, op0=OP.add)

                for (tt0, tp) in _ntiles(gn, 128):
                    x3_tm = wk2.tile([128, DIM], f32, tag="x3_tm", bufs=3)
                    nc.sync.dma_start(out=x3_tm[:tp, :],
                                      in_=xaap[g0 + tt0: g0 + tt0 + tp, :])
                    out_tm = wk2.tile([128, DIM], f32, tag="out_tm", bufs=2)
                    for k in range(KC):
                        tb = ps2.tile([128, 128], bf16, tag="tp", bufs=2,
                                      name="tb2")
                        nc.tensor.transpose(tb[:tp, :],
                                            mlpf[:, k, tt0:tt0 + tp],
                                            identb[:, :])
                        nc.vector.tensor_add(
                            out=out_tm[:tp, k * 128:(k + 1) * 128],
                            in0=tb[:tp, :],
                            in1=x3_tm[:tp, k * 128:(k + 1) * 128])
                    nc.sync.dma_start(out=xoap[g0 + tt0: g0 + tt0 + tp, :],
                                      in_=out_tm[:tp, :])

    nc.compile()
    return nc


def _bilinear_7to8(t):
    # (..., 7, 7) -> (..., 8, 8), matches F.interpolate(bilinear, align_corners=False)
    src = np.clip((np.arange(8, dtype=np.float32) + 0.5) * (7.0 / 8.0) - 0.5,
                  0.0, None)
    i0 = np.floor(src).astype(np.int64)
    i1 = np.minimum(i0 + 1, 6)
    f = src - i0

    def lerp(x, axis):
        a = np.take(x, i0, axis=axis)
        b = np.take(x, i1, axis=axis)
        shp = [1] * x.ndim
        shp[axis] = 8
        ff = f.reshape(shp)
        return a * (1.0 - ff) + b * ff

    return lerp(lerp(t, -2), -1)


def _fm_vec(v, chunks):
    # [chunks*128] feature vector -> [128, chunks] feature-major tile layout
    return np.ascontiguousarray(
        np.asarray(v, dtype=np.float32).reshape(chunks, 128).T)


def kernel(**inputs):
    global _PROG
    from concourse.bass_utils import run_bass_kernel_spmd

    if _PROG is None:
        _PROG = _build_program()
    nc = _PROG

    f = {k: np.asarray(v, dtype=np.float32) for k, v in inputs.items()}
    x = f["x"]

    wqkvT = np.ascontiguousarray(f["w_qkv"].T)
    wprojT = np.ascontiguousarray(f["w_proj"].T)
    fc1T = np.ascontiguousarray(f["fc1_w"].T)
    fc2T = np.ascontiguousarray(f["fc2_w"].T)

    # agent->token bias pb [49, 65] and token->agent bias ab [65, 49]
    pb1 = _bilinear_7to8(f["an_bias"]).reshape(1, AG, 64)
    pb2 = (f["ah_bias"] + f["aw_bias"]).reshape(1, AG, 64)
    pb = np.concatenate([f["ac_bias"].reshape(1, AG, 1), pb1 + pb2], axis=-1)
    pbc = np.ascontiguousarray(pb[0] / SCALE4)

    ab1 = _bilinear_7to8(f["na_bias"]).reshape(1, AG, 64).transpose(0, 2, 1)
    ab2 = (f["ha_bias"] + f["wa_bias"]).reshape(1, 64, AG)
    ab = np.concatenate([f["ca_bias"].reshape(1, 1, AG), ab1 + ab2], axis=-2)
    abc = np.ascontiguousarray(ab[0] / SCALE4)

    dwcw = np.ascontiguousarray(
        f["dwc_w"].reshape(DIM, 9).reshape(KC, 128, 9).transpose(1, 0, 2)
    ).reshape(128, KC * 9)

    common = {
        "wqkvT": wqkvT, "wprojT": wprojT, "fc1T": fc1T, "fc2T": fc2T,
        "ln1g": _fm_vec(f["ln1_g"], KC), "ln1b": _fm_vec(f["ln1_b"], KC),
        "ln2g": _fm_vec(f["ln2_g"], KC), "ln2b": _fm_vec(f["ln2_b"], KC),
        "bproj": _fm_vec(f["b_proj"], KC),
        "fc1b": _fm_vec(f["fc1_b"], MC), "fc2b": _fm_vec(f["fc2_b"], KC),
        "dwcw": dwcw, "dwcb": _fm_vec(f["dwc_b"], KC),
        "pbc": pbc.astype(np.float32), "abc": abc.astype(np.float32),
    }
    xs = x.reshape(NCORES, NITEMS * NTOK, DIM)
    in_maps = [dict(common, x=np.ascontiguousarray(xs[c]))
               for c in range(NCORES)]

    res = run_bass_kernel_spmd(nc, in_maps, list(range(NCORES)))

    xout = np.stack([res.results[c]["xout"] for c in range(NCORES)])
    xout = xout.reshape(NCORES * NITEMS, NTOK, DIM)
    rep = np.stack([res.results[c]["rep"] for c in range(NCORES)])
    rep = rep.reshape(NCORES * NITEMS, 1, AG, NTOK)
    return xout.astype(np.float32), rep.astype(np.float32)
